# revision 11
# baseline (speedup 1.0000x reference)
"""Trainium2 Bass kernel for the interval-bound dual Conv2D problem.

Problem (hardcoded shapes):
  lx_in, ux_in : (8, 56, 56, 28, 28) f32   symbolic coeff maps
  lc_in, uc_in : (8, 56, 56) f32           constant offset maps
  weight       : (16, 8, 3, 3) f32, bias : (16,) f32
  outputs: lx_out/ux_out (16, 56, 56, 28, 28), lc_out/uc_out (16, 56, 56)

Strategy
--------
The trailing (28, 28) symbolic dims form a pure conv batch of 784, sharded
98-per-core across 8 NeuronCores (data parallel, no collectives). The lc/uc
constant maps ride along as a 99th batch column on core 0 (bias added on
host); other cores pad with zeros.

Per core, the dual 3x3 conv is computed as a banded matmul:
  contraction partitions (s=2, wi=6, ci=8) = 96   [s: {lx,ux} input]
  output partitions      (wo=4, t=2, co=16) = 128 [t: {lower,upper} output]
The stationary lhsT (one per kh tap) holds the sign-split weights
(w_pos/w_neg) as a kw-band; both interval outputs come out of one PSUM
tile. The 3 kh taps are 3 accumulating matmuls over shifted (h, b) slices
of the same SBUF tile, so there is no im2col materialization. Matmuls run
as float32r (full-rate fp32 path, N=396 >= 256).

Host-side layouts are chosen so that every DMA collapses to <=3 dims with
large contiguous runs:
  xs  : (2, 58, 8, 58, 99)  (s, w_pad, ci, h_pad, b) - each w-tile load is
        one contiguous 1.1 MB block per s (zero-padded borders, no edge
        cases on device)
  out : (56, 2, 16, 56, 99) (w, t, co, h, b) - each w-tile store is one
        fully contiguous 2.8 MB block over all 128 partitions
"""

import numpy as np

import concourse.bacc as bacc
import concourse.bass as bass
import concourse.mybir as mybir
import concourse.tile as tile
from concourse.bass_utils import run_bass_kernel_spmd

# Geometry (hardcoded per spec)
CIN, COUT, H, W, K = 8, 16, 56, 56, 3
S = 28
B_ALL = S * S            # 784 conv batch
NCORES = 8
BC = B_ALL // NCORES     # 98 batch cols per core
B = BC + 1               # +1 col: const maps (core 0) / zeros (others)
WO, WI = 4, 6            # w-tile output/input width
HT = 4                   # h-stripe height -> matmul N = HT*B = 396
NWT = W // WO            # 14 w-tiles
NHT = H // HT            # 14 h-stripes
NP_IN = 2 * WI * CIN     # 96 contraction partitions
NP_OUT = WO * 2 * COUT   # 128 output partitions

F32 = mybir.dt.float32
F32R = mybir.dt.float32r

_CACHED_NC = None


def _build_nc():
    nc = bacc.Bacc(trn_type="TRN2", target_bir_lowering=False, debug=False)

    xs = nc.dram_tensor("xs", [2, W + 2, CIN, H + 2, B], F32R, kind="ExternalInput")
    wt = nc.dram_tensor("wt", [NP_IN, K, NP_OUT], F32R, kind="ExternalInput")
    out = nc.dram_tensor("out", [W, 2, COUT, H, B], F32, kind="ExternalOutput")

    xs_ap, wt_ap, out_ap = xs.ap(), wt.ap(), out.ap()

    with tile.TileContext(nc) as tc:
        with (
            tc.tile_pool(name="wpool", bufs=1) as wpool,
            tc.tile_pool(name="xpool", bufs=2) as xpool,
            tc.tile_pool(name="opool", bufs=2) as opool,
            tc.tile_pool(name="pspool", bufs=4, space="PSUM") as pspool,
        ):
            w_sb = wpool.tile([NP_IN, K * NP_OUT], F32R)
            nc.sync.dma_start(w_sb[:], wt_ap[:])

            for j in range(NWT):
                x_sb = xpool.tile([NP_IN, (H + 2) * B], F32R)
                nc.sync.dma_start(x_sb[:], xs_ap[:, WO * j:WO * j + WI])

                o_sb = opool.tile([NP_OUT, H * B], F32)
                for i in range(NHT):
                    ps = pspool.tile([NP_OUT, HT * B], F32)
                    for kh in range(K):
                        nc.tensor.matmul(
                            ps[:],
                            w_sb[:, kh * NP_OUT:(kh + 1) * NP_OUT],
                            x_sb[:, (HT * i + kh) * B:(HT * i + kh + HT) * B],
                            start=(kh == 0),
                            stop=(kh == K - 1),
                        )
                    nc.vector.tensor_copy(o_sb[:, i * HT * B:(i + 1) * HT * B], ps[:])

                nc.scalar.dma_start(out_ap[WO * j:WO * j + WO], o_sb[:])

    nc.compile()
    return nc


def _get_nc():
    global _CACHED_NC
    if _CACHED_NC is None:
        _CACHED_NC = _build_nc()
    return _CACHED_NC


def _build_whost(weight):
    """lhsT bank: (96, 3, 128): [p=(s,wi,ci), kh, m=(wo,t,co)]."""
    w_pos = np.where(weight > 0, weight, 0.0).astype(np.float32)
    w_neg = np.where(weight < 0, weight, 0.0).astype(np.float32)
    wh = np.zeros((2, WI, CIN, K, WO, 2, COUT), np.float32)
    for s in range(2):
        for t in range(2):
            wst = (w_pos if s == t else w_neg).transpose(1, 2, 3, 0)  # (ci,kh,kw,co)
            for wo in range(WO):
                for kw in range(K):
                    wh[s, wo + kw, :, :, wo, t, :] = wst[:, :, kw, :]
    return np.ascontiguousarray(wh.reshape(NP_IN, K, NP_OUT))


def _pack_inputs(lx, ux, lc, uc):
    """Build per-core xs arrays: (2, 58, 8, 58, 99), zero-padded borders."""
    maps = []
    for c in range(NCORES):
        xs = np.zeros((2, W + 2, CIN, H + 2, B), np.float32)
        sl = slice(BC * c, BC * (c + 1))
        # (ci,h,w,b) -> (w,ci,h,b)
        xs[0, 1:W + 1, :, 1:H + 1, :BC] = lx[..., sl].transpose(2, 0, 1, 3)
        xs[1, 1:W + 1, :, 1:H + 1, :BC] = ux[..., sl].transpose(2, 0, 1, 3)
        if c == 0:
            xs[0, 1:W + 1, :, 1:H + 1, BC] = lc.transpose(2, 0, 1)
            xs[1, 1:W + 1, :, 1:H + 1, BC] = uc.transpose(2, 0, 1)
        maps.append(xs)
    return maps


def _unpack_outputs(results, bias):
    lxo = np.empty((COUT, H, W, B_ALL), np.float32)
    uxo = np.empty((COUT, H, W, B_ALL), np.float32)
    lco = uco = None
    for c in range(NCORES):
        o = results[c]["out"]  # (w, t, co, h, b)
        sl = slice(BC * c, BC * (c + 1))
        lxo[..., sl] = o[:, 0, :, :, :BC].transpose(1, 2, 0, 3)
        uxo[..., sl] = o[:, 1, :, :, :BC].transpose(1, 2, 0, 3)
        if c == 0:
            lco = o[:, 0, :, :, BC].transpose(1, 2, 0) + bias[:, None, None]
            uco = o[:, 1, :, :, BC].transpose(1, 2, 0) + bias[:, None, None]
    return (
        lxo.reshape(COUT, H, W, S, S),
        uxo.reshape(COUT, H, W, S, S),
        np.ascontiguousarray(lco),
        np.ascontiguousarray(uco),
    )


def run_kernel(inputs, trace=False):
    lx = np.asarray(inputs["lx_in"], np.float32).reshape(CIN, H, W, B_ALL)
    ux = np.asarray(inputs["ux_in"], np.float32).reshape(CIN, H, W, B_ALL)
    lc = np.asarray(inputs["lc_in"], np.float32)
    uc = np.asarray(inputs["uc_in"], np.float32)
    weight = np.asarray(inputs["weight"], np.float32)
    bias = np.asarray(inputs["bias"], np.float32)

    wh = _build_whost(weight)
    in_maps = [{"xs": xs, "wt": wh} for xs in _pack_inputs(lx, ux, lc, uc)]

    nc = _get_nc()
    res = run_bass_kernel_spmd(nc, in_maps, core_ids=list(range(NCORES)), trace=trace)
    return _unpack_outputs(res.results, bias), res


def kernel(**inputs):
    outs, _ = run_kernel(inputs, trace=False)
    return outs


# revision 13
# speedup vs baseline: 2.7455x; 2.7455x over previous
"""Trainium2 Bass kernel for the interval-bound dual Conv2D problem.

Problem (hardcoded shapes):
  lx_in, ux_in : (8, 56, 56, 28, 28) f32   symbolic coeff maps
  lc_in, uc_in : (8, 56, 56) f32           constant offset maps
  weight       : (16, 8, 3, 3) f32, bias : (16,) f32
  outputs: lx_out/ux_out (16, 56, 56, 28, 28), lc_out/uc_out (16, 56, 56)

Strategy
--------
The trailing (28, 28) symbolic dims form a pure conv batch of 784, sharded
98-per-core across 8 NeuronCores (data parallel, no collectives). The lc/uc
constant maps ride along as a 99th batch column on core 0 (bias added on
host); other cores pad with zeros.

Per core, the dual 3x3 conv is computed as a banded matmul:
  contraction partitions (s=2, wi=6, ci=8) = 96   [s: {lx,ux} input]
  output partitions      (wo=4, t=2, co=16) = 128 [t: {lower,upper} output]
The stationary lhsT (one per kh tap) holds the sign-split weights
(w_pos/w_neg) as a kw-band; both interval outputs come out of one PSUM
tile. The 3 kh taps are 3 accumulating matmuls over shifted (h, b) slices
of the same SBUF tile, so there is no im2col materialization. Matmuls run
as float32r (full-rate fp32 path, N=396 >= 256).

Host-side layouts are chosen so that every DMA collapses to <=3 dims with
large contiguous runs:
  xs  : (2, 58, 8, 58, 99)  (s, w_pad, ci, h_pad, b) - each w-tile load is
        one contiguous 1.1 MB block per s (zero-padded borders, no edge
        cases on device)
  out : (56, 2, 16, 56, 99) (w, t, co, h, b) - each w-tile store is one
        fully contiguous 2.8 MB block over all 128 partitions
"""

import numpy as np

import concourse.bacc as bacc
import concourse.bass as bass
import concourse.mybir as mybir
import concourse.tile as tile
from concourse.bass_utils import run_bass_kernel_spmd

# Geometry (hardcoded per spec)
CIN, COUT, H, W, K = 8, 16, 56, 56, 3
S = 28
B_ALL = S * S            # 784 conv batch
NCORES = 8
BC = B_ALL // NCORES     # 98 batch cols per core
B = BC + 1               # +1 col: const maps (core 0) / zeros (others)
WO, WI = 4, 6            # w-tile output/input width
HT = 4                   # h-stripe height -> matmul N = HT*B = 396
NWT = W // WO            # 14 w-tiles
NHT = H // HT            # 14 h-stripes
NP_IN = 2 * WI * CIN     # 96 contraction partitions
NP_OUT = WO * 2 * COUT   # 128 output partitions

F32 = mybir.dt.float32
F32R = mybir.dt.float32r

_CACHED_NC = None


def _build_nc():
    nc = bacc.Bacc(trn_type="TRN2", target_bir_lowering=False, debug=False)

    xs = nc.dram_tensor("xs", [2, W + 2, CIN, H + 2, B], F32R, kind="ExternalInput")
    wt = nc.dram_tensor("wt", [NP_IN, K, NP_OUT], F32R, kind="ExternalInput")
    out = nc.dram_tensor("out", [W, 2, COUT, H, B], F32, kind="ExternalOutput")

    xs_ap, wt_ap, out_ap = xs.ap(), wt.ap(), out.ap()

    with tile.TileContext(nc) as tc:
        with (
            tc.tile_pool(name="wpool", bufs=1) as wpool,
            tc.tile_pool(name="xpool", bufs=2) as xpool,
            tc.tile_pool(name="opool", bufs=2) as opool,
            tc.tile_pool(name="pspool", bufs=4, space="PSUM") as pspool,
        ):
            w_sb = wpool.tile([NP_IN, K * NP_OUT], F32R)
            nc.sync.dma_start(w_sb[:], wt_ap[:])

            # Iterate the input in (w, ci, s, h*b) order: the HWDGE ring
            # distributes descriptors over the 16 SDMA engines by the
            # OUTERMOST AP dim, so the merged (w ci)=48 outer dim gives
            # 16-way engine parallelism ((s ...)=2 outer uses only 2).
            xs_t = xs_ap.transpose([1, 2, 0, 3, 4])
            for j in range(NWT):
                x_sb = xpool.tile([NP_IN, (H + 2) * B], F32R)
                nc.sync.dma_start(x_sb[:], xs_t[WO * j:WO * j + WI])

                o_sb = opool.tile([NP_OUT, H * B], F32)
                for i in range(NHT):
                    ps = pspool.tile([NP_OUT, HT * B], F32)
                    for kh in range(K):
                        nc.tensor.matmul(
                            ps[:],
                            w_sb[:, kh * NP_OUT:(kh + 1) * NP_OUT],
                            x_sb[:, (HT * i + kh) * B:(HT * i + kh + HT) * B],
                            start=(kh == 0),
                            stop=(kh == K - 1),
                        )
                    nc.vector.tensor_copy(o_sb[:, i * HT * B:(i + 1) * HT * B], ps[:])

                nc.scalar.dma_start(out_ap[WO * j:WO * j + WO], o_sb[:])

    nc.compile()
    return nc


def _get_nc():
    global _CACHED_NC
    if _CACHED_NC is None:
        _CACHED_NC = _build_nc()
    return _CACHED_NC


def _build_whost(weight):
    """lhsT bank: (96, 3, 128): [p=(wi,ci,s), kh, m=(wo,t,co)]."""
    w_pos = np.where(weight > 0, weight, 0.0).astype(np.float32)
    w_neg = np.where(weight < 0, weight, 0.0).astype(np.float32)
    wh = np.zeros((WI, CIN, 2, K, WO, 2, COUT), np.float32)
    for s in range(2):
        for t in range(2):
            wst = (w_pos if s == t else w_neg).transpose(1, 2, 3, 0)  # (ci,kh,kw,co)
            for wo in range(WO):
                for kw in range(K):
                    wh[wo + kw, :, s, :, wo, t, :] = wst[:, :, kw, :]
    return np.ascontiguousarray(wh.reshape(NP_IN, K, NP_OUT))


def _pack_inputs(lx, ux, lc, uc):
    """Build per-core xs arrays: (2, 58, 8, 58, 99), zero-padded borders."""
    maps = []
    for c in range(NCORES):
        xs = np.zeros((2, W + 2, CIN, H + 2, B), np.float32)
        sl = slice(BC * c, BC * (c + 1))
        # (ci,h,w,b) -> (w,ci,h,b)
        xs[0, 1:W + 1, :, 1:H + 1, :BC] = lx[..., sl].transpose(2, 0, 1, 3)
        xs[1, 1:W + 1, :, 1:H + 1, :BC] = ux[..., sl].transpose(2, 0, 1, 3)
        if c == 0:
            xs[0, 1:W + 1, :, 1:H + 1, BC] = lc.transpose(2, 0, 1)
            xs[1, 1:W + 1, :, 1:H + 1, BC] = uc.transpose(2, 0, 1)
        maps.append(xs)
    return maps


def _unpack_outputs(results, bias):
    lxo = np.empty((COUT, H, W, B_ALL), np.float32)
    uxo = np.empty((COUT, H, W, B_ALL), np.float32)
    lco = uco = None
    for c in range(NCORES):
        o = results[c]["out"]  # (w, t, co, h, b)
        sl = slice(BC * c, BC * (c + 1))
        lxo[..., sl] = o[:, 0, :, :, :BC].transpose(1, 2, 0, 3)
        uxo[..., sl] = o[:, 1, :, :, :BC].transpose(1, 2, 0, 3)
        if c == 0:
            lco = o[:, 0, :, :, BC].transpose(1, 2, 0) + bias[:, None, None]
            uco = o[:, 1, :, :, BC].transpose(1, 2, 0) + bias[:, None, None]
    return (
        lxo.reshape(COUT, H, W, S, S),
        uxo.reshape(COUT, H, W, S, S),
        np.ascontiguousarray(lco),
        np.ascontiguousarray(uco),
    )


def run_kernel(inputs, trace=False):
    lx = np.asarray(inputs["lx_in"], np.float32).reshape(CIN, H, W, B_ALL)
    ux = np.asarray(inputs["ux_in"], np.float32).reshape(CIN, H, W, B_ALL)
    lc = np.asarray(inputs["lc_in"], np.float32)
    uc = np.asarray(inputs["uc_in"], np.float32)
    weight = np.asarray(inputs["weight"], np.float32)
    bias = np.asarray(inputs["bias"], np.float32)

    wh = _build_whost(weight)
    in_maps = [{"xs": xs, "wt": wh} for xs in _pack_inputs(lx, ux, lc, uc)]

    nc = _get_nc()
    res = run_bass_kernel_spmd(nc, in_maps, core_ids=list(range(NCORES)), trace=trace)
    return _unpack_outputs(res.results, bias), res


def kernel(**inputs):
    outs, _ = run_kernel(inputs, trace=False)
    return outs


# revision 14
# speedup vs baseline: 3.3044x; 1.2036x over previous
"""Trainium2 Bass kernel for the interval-bound dual Conv2D problem.

Problem (hardcoded shapes):
  lx_in, ux_in : (8, 56, 56, 28, 28) f32   symbolic coeff maps
  lc_in, uc_in : (8, 56, 56) f32           constant offset maps
  weight       : (16, 8, 3, 3) f32, bias : (16,) f32
  outputs: lx_out/ux_out (16, 56, 56, 28, 28), lc_out/uc_out (16, 56, 56)

Strategy
--------
The trailing (28, 28) symbolic dims form a pure conv batch of 784, sharded
98-per-core across 8 NeuronCores (data parallel, no collectives). The lc/uc
constant maps ride along as a 99th batch column on core 0 (bias added on
host); other cores pad with zeros.

Per core, the dual 3x3 conv is computed as a banded matmul:
  contraction partitions (s=2, wi=6, ci=8) = 96   [s: {lx,ux} input]
  output partitions      (wo=4, t=2, co=16) = 128 [t: {lower,upper} output]
The stationary lhsT (one per kh tap) holds the sign-split weights
(w_pos/w_neg) as a kw-band; both interval outputs come out of one PSUM
tile. The 3 kh taps are 3 accumulating matmuls over shifted (h, b) slices
of the same SBUF tile, so there is no im2col materialization. Matmuls run
as float32r (full-rate fp32 path, N=396 >= 256).

Host-side layouts are chosen so that every DMA collapses to <=3 dims with
large contiguous runs:
  xs  : (2, 58, 8, 58, 99)  (s, w_pad, ci, h_pad, b) - each w-tile load is
        one contiguous 1.1 MB block per s (zero-padded borders, no edge
        cases on device)
  out : (56, 2, 16, 56, 99) (w, t, co, h, b) - each w-tile store is one
        fully contiguous 2.8 MB block over all 128 partitions
"""

import numpy as np

import concourse.bacc as bacc
import concourse.bass as bass
import concourse.mybir as mybir
import concourse.tile as tile
from concourse.bass_utils import run_bass_kernel_spmd

# Geometry (hardcoded per spec)
CIN, COUT, H, W, K = 8, 16, 56, 56, 3
S = 28
B_ALL = S * S            # 784 conv batch
NCORES = 8
BC = B_ALL // NCORES     # 98 batch cols per core
B = BC + 1               # +1 col: const maps (core 0) / zeros (others)
WO, WI = 4, 6            # w-tile output/input width
HT = 4                   # h-stripe height -> matmul N = HT*B = 396
NWT = W // WO            # 14 w-tiles
NHT = H // HT            # 14 h-stripes
NP_IN = 2 * WI * CIN     # 96 contraction partitions
NP_OUT = WO * 2 * COUT   # 128 output partitions

F32 = mybir.dt.float32
F32R = mybir.dt.float32r

_CACHED_NC = None


def _build_nc():
    nc = bacc.Bacc(trn_type="TRN2", target_bir_lowering=False, debug=False)

    xs = nc.dram_tensor("xs", [2, W + 2, CIN, H + 2, B], F32R, kind="ExternalInput")
    wt = nc.dram_tensor("wt", [NP_IN, K, NP_OUT], F32R, kind="ExternalInput")
    out = nc.dram_tensor("out", [W, 2, COUT, H, B], F32, kind="ExternalOutput")

    xs_ap, wt_ap, out_ap = xs.ap(), wt.ap(), out.ap()

    with tile.TileContext(nc) as tc:
        with (
            tc.tile_pool(name="wpool", bufs=1) as wpool,
            tc.tile_pool(name="xpool", bufs=3) as xpool,
            tc.tile_pool(name="opool", bufs=3) as opool,
            tc.tile_pool(name="pspool", bufs=6, space="PSUM") as pspool,
        ):
            w_sb = wpool.tile([NP_IN, K * NP_OUT], F32R)
            nc.sync.dma_start(w_sb[:], wt_ap[:])

            # Iterate the input in (w, ci, s, h*b) order: the HWDGE ring
            # distributes descriptors over the 16 SDMA engines by the
            # OUTERMOST AP dim, so the merged (w ci)=48 outer dim gives
            # 16-way engine parallelism ((s ...)=2 outer uses only 2).
            xs_t = xs_ap.transpose([1, 2, 0, 3, 4])
            for j in range(NWT):
                x_sb = xpool.tile([NP_IN, (H + 2) * B], F32R)
                nc.sync.dma_start(x_sb[:], xs_t[WO * j:WO * j + WI])

                o_sb = opool.tile([NP_OUT, H * B], F32)
                for i in range(NHT):
                    ps = pspool.tile([NP_OUT, HT * B], F32)
                    for kh in range(K):
                        nc.tensor.matmul(
                            ps[:],
                            w_sb[:, kh * NP_OUT:(kh + 1) * NP_OUT],
                            x_sb[:, (HT * i + kh) * B:(HT * i + kh + HT) * B],
                            start=(kh == 0),
                            stop=(kh == K - 1),
                        )
                    nc.vector.tensor_copy(o_sb[:, i * HT * B:(i + 1) * HT * B], ps[:])

                nc.scalar.dma_start(out_ap[WO * j:WO * j + WO], o_sb[:])

    nc.compile()
    return nc


def _get_nc():
    global _CACHED_NC
    if _CACHED_NC is None:
        _CACHED_NC = _build_nc()
    return _CACHED_NC


def _build_whost(weight):
    """lhsT bank: (96, 3, 128): [p=(wi,ci,s), kh, m=(wo,t,co)]."""
    w_pos = np.where(weight > 0, weight, 0.0).astype(np.float32)
    w_neg = np.where(weight < 0, weight, 0.0).astype(np.float32)
    wh = np.zeros((WI, CIN, 2, K, WO, 2, COUT), np.float32)
    for s in range(2):
        for t in range(2):
            wst = (w_pos if s == t else w_neg).transpose(1, 2, 3, 0)  # (ci,kh,kw,co)
            for wo in range(WO):
                for kw in range(K):
                    wh[wo + kw, :, s, :, wo, t, :] = wst[:, :, kw, :]
    return np.ascontiguousarray(wh.reshape(NP_IN, K, NP_OUT))


def _pack_inputs(lx, ux, lc, uc):
    """Build per-core xs arrays: (2, 58, 8, 58, 99), zero-padded borders."""
    maps = []
    for c in range(NCORES):
        xs = np.zeros((2, W + 2, CIN, H + 2, B), np.float32)
        sl = slice(BC * c, BC * (c + 1))
        # (ci,h,w,b) -> (w,ci,h,b)
        xs[0, 1:W + 1, :, 1:H + 1, :BC] = lx[..., sl].transpose(2, 0, 1, 3)
        xs[1, 1:W + 1, :, 1:H + 1, :BC] = ux[..., sl].transpose(2, 0, 1, 3)
        if c == 0:
            xs[0, 1:W + 1, :, 1:H + 1, BC] = lc.transpose(2, 0, 1)
            xs[1, 1:W + 1, :, 1:H + 1, BC] = uc.transpose(2, 0, 1)
        maps.append(xs)
    return maps


def _unpack_outputs(results, bias):
    lxo = np.empty((COUT, H, W, B_ALL), np.float32)
    uxo = np.empty((COUT, H, W, B_ALL), np.float32)
    lco = uco = None
    for c in range(NCORES):
        o = results[c]["out"]  # (w, t, co, h, b)
        sl = slice(BC * c, BC * (c + 1))
        lxo[..., sl] = o[:, 0, :, :, :BC].transpose(1, 2, 0, 3)
        uxo[..., sl] = o[:, 1, :, :, :BC].transpose(1, 2, 0, 3)
        if c == 0:
            lco = o[:, 0, :, :, BC].transpose(1, 2, 0) + bias[:, None, None]
            uco = o[:, 1, :, :, BC].transpose(1, 2, 0) + bias[:, None, None]
    return (
        lxo.reshape(COUT, H, W, S, S),
        uxo.reshape(COUT, H, W, S, S),
        np.ascontiguousarray(lco),
        np.ascontiguousarray(uco),
    )


def run_kernel(inputs, trace=False):
    lx = np.asarray(inputs["lx_in"], np.float32).reshape(CIN, H, W, B_ALL)
    ux = np.asarray(inputs["ux_in"], np.float32).reshape(CIN, H, W, B_ALL)
    lc = np.asarray(inputs["lc_in"], np.float32)
    uc = np.asarray(inputs["uc_in"], np.float32)
    weight = np.asarray(inputs["weight"], np.float32)
    bias = np.asarray(inputs["bias"], np.float32)

    wh = _build_whost(weight)
    in_maps = [{"xs": xs, "wt": wh} for xs in _pack_inputs(lx, ux, lc, uc)]

    nc = _get_nc()
    res = run_bass_kernel_spmd(nc, in_maps, core_ids=list(range(NCORES)), trace=trace)
    return _unpack_outputs(res.results, bias), res


def kernel(**inputs):
    outs, _ = run_kernel(inputs, trace=False)
    return outs
